# revision 26
# baseline (speedup 1.0000x reference)
"""Trainium2 Bass kernel for nn_CrossGraphNetLite (dual-GNN + gated fusion + classifier).

Strategy (8 NeuronCores, graph/data parallel, fp8 streams):
  * Host preprocesses the integer graph structure into dense coefficient
    matrices, quantized to fp8 e4m3:
      - Layer 1 per dst-node block:  T[t, v] = sum of edge coeffs into v
        bucketed by source-node *type* t (+ self-loop + bias row). On device
        x2 = relu(ea^T T) with ea = [emb @ W1; b1] in fp16.
      - Layer 2 + mean-pool collapse: C[s, g] = sum of edge coeffs from src s
        (this core's block) into any node of graph g (+ self-loop), fp8.
        pool^T += h2[pair]^T C[pair] in DoubleRow fp8 perf mode.
  * C tiles stream on the sync+scalar HWDGE queues as [128, 4096] granules
    (2 KB contiguous per-partition descriptors); T streams on gpsimd SWDGE.
  * The cross-core reduction avoids the ~26 us RDH ReduceScatter: C's graph
    columns are XOR-permuted per core (graph block b sits at position b^core)
    so identical SPMD remote_dma_broadcast instructions with *relative* dests
    deliver exactly the receiver's 128 graphs: core m sends its position-s
    slice with delta s to core m^s. Receivers sum 8 slices locally.
  * Tight epilogue: hA-hC / hC extracted via [I;-I],[0;I] identity matmuls
    (no cross-partition DVE), LayerNorm folded into the classifier weights.
"""

import sys

sys.path.insert(0, "/opt/trn_rl_repo")

import numpy as np
import ml_dtypes

import concourse.bacc as bacc
import concourse.bass as bass
import concourse.mybir as mybir
import concourse.tile as tile

AF = mybir.ActivationFunctionType
ALU = mybir.AluOpType
PM = mybir.MatmulPerfMode
F32 = mybir.dt.float32
F16 = mybir.dt.float16
F8 = mybir.dt.float8e4
NP_F8 = ml_dtypes.float8_e4m3

USE_P2P = False


class CFG:
    def __init__(self):
        self.N = 100000
        self.E = 1250000
        self.G = 1024
        self.NCORES = 8
        self.NTA = 200                             # + bias row -> 201, pad 202
        self.NTC = 100                             # + bias row -> 101, pad 102
        self.NTA2 = 202
        self.NTC2 = 102
        self.SEM = 768
        self.NB = self.N // self.NCORES            # 12500
        self.NBP = 12800                           # 25 x 512 (x2T width)
        self.NBPT = 13312                          # 13 x 1024 (T storage)
        self.NCHUNK = 25                           # 512-node chunks
        self.NPAIR = 50                            # 256-node pairs (49 real)
        self.NGRAN = 25                            # C granules (2 pairs each)
        self.TGRAN = 13                            # T granules (2 chunks each)
        self.GB = self.G // self.NCORES            # 128
        self.SEMK = 6
        self.SPLIT = 35                            # sem branch after pair 34
        self.PREF_C = 10                           # C granule prefetch depth


def build_nc(cfg: CFG):
    nc = bacc.Bacc("TRN2", target_bir_lowering=False, debug=False,
                   enable_asserts=False, num_devices=cfg.NCORES,
                   num_swdge_queues=4)
    G, GB = cfg.G, cfg.GB
    RG = [list(range(cfg.NCORES))]

    def din(name, shape, dt=F32):
        return nc.dram_tensor(name, list(shape), dt, kind="ExternalInput").ap()

    T_a0 = din("T_a0", [cfg.TGRAN, 128, 1024], F8)
    T_a1 = din("T_a1", [cfg.TGRAN, 74, 1024], F8)
    T_c = din("T_c", [cfg.TGRAN, 102, 1024], F8)
    # granule-packed: [q, p, 4096] = pairs (2q, 2q+1) contiguous per
    # partition -> one 4 KB DMA descriptor per partition
    C_ast = din("C_ast", [cfg.NGRAN, 128, 4096], F8)
    C_cfg = din("C_cfg", [cfg.NGRAN, 128, 4096], F8)
    embT_ast = din("embT_ast", [64, cfg.NTA2])
    embT_cfg = din("embT_cfg", [64, cfg.NTC2])
    astW1 = din("astW1", [64, 64])
    cfgW1 = din("cfgW1", [64, 64])
    astb1h = din("astb1h", [1, 64], F16)
    cfgb1h = din("cfgb1h", [1, 64], F16)
    W2blk = din("W2blk", [128, 128], F16)
    astb2h = din("astb2h", [1, 64], F16)
    cfgb2h = din("cfgb2h", [1, 64], F16)
    cnt_ast = din("cnt_ast", [1, G], F16)
    cnt_cfg = din("cnt_cfg", [1, G], F16)
    Wg1h = din("Wg1h", [128, 64], F16)
    bg1c = din("bg1c", [64, 1])
    Wsemh = din("Wsemh", [cfg.SEM, 64], F16)
    bsemc = din("bsemc", [64, 1])
    semTh = din("semTh", [cfg.SEM, GB], F16)
    Wg2h = din("Wg2h", [128, 64], F16)
    bg2c = din("bg2c", [64, 1])
    Wcph = din("Wcph", [64, 2], F16)
    bcp = din("bcp", [2, 1])
    DD = din("DD", [128, 64], F16)                 # [I; -I]
    DH = din("DH", [128, 64], F16)                 # [0; I]
    out_ap = nc.dram_tensor("outT", [2, GB], F32, kind="ExternalOutput").ap()

    if USE_P2P:
        p2p_sem = nc.alloc_semaphore("p2p_arrival")
        p2p_loc = nc.alloc_semaphore("p2p_local")
    patches = []

    with tile.TileContext(nc) as tc:
        with (
            tc.tile_pool(name="consts", bufs=1) as consts,
            tc.tile_pool(name="x2t", bufs=1) as x2t_pool,
            tc.tile_pool(name="ta0", bufs=5) as ta0_pool,
            tc.tile_pool(name="ta1", bufs=5) as ta1_pool,
            tc.tile_pool(name="tcc", bufs=5) as tcc_pool,
            tc.tile_pool(name="ca", bufs=12) as ca_pool,
            tc.tile_pool(name="cc", bufs=12) as cc_pool,
            tc.tile_pool(name="h2p", bufs=8) as h2p,
            tc.tile_pool(name="small", bufs=1) as small,
            tc.tile_pool(name="ps_px", bufs=2, space="PSUM") as ps_px,
            tc.tile_pool(name="ps_ph", bufs=2, space="PSUM") as ps_ph,
            tc.tile_pool(name="ps_pool", bufs=1, space="PSUM") as ps_pool,
            tc.tile_pool(name="dram", bufs=1, space="DRAM") as dram,
        ):
            # ---- critical consts: sync feeds the ea build, scalar feeds
            # W2/cnt/b2 (needed within a few us) ----
            def load_c(eng, ap, shape, dt=F32, name=None):
                t = consts.tile(list(shape), dt, name=name or ap.tensor.name + "_sb")
                eng.dma_start(t[:], ap[:])
                return t

            embT_ast_sb = load_c(nc.sync, embT_ast, [64, cfg.NTA2])
            embT_cfg_sb = load_c(nc.sync, embT_cfg, [64, cfg.NTC2])
            astW1_sb = load_c(nc.sync, astW1, [64, 64])
            cfgW1_sb = load_c(nc.sync, cfgW1, [64, 64])
            W2blk_sb = load_c(nc.scalar, W2blk, [128, 128], F16)
            astb2_sb = load_c(nc.scalar, astb2h, [1, 64], F16)
            cfgb2_sb = load_c(nc.scalar, cfgb2h, [1, 64], F16)
            cnt_ast_sb = load_c(nc.scalar, cnt_ast, [1, G], F16)
            cnt_cfg_sb = load_c(nc.scalar, cnt_cfg, [1, G], F16)

            # ---- ea tables: [emb @ W1 ; b1] in fp16 ----
            def build_ea(embT_sb, W1_sb, b1_ap, ksl, brow, tag):
                tiles = []
                for i, (k0, k1) in enumerate(ksl):
                    kw = k1 - k0
                    ps = ps_ph.tile([kw, 64], F32, name=f"psea_{tag}{i}", tag="ph")
                    nc.tensor.matmul(ps[:], embT_sb[:, k0:k1], W1_sb[:],
                                     start=True, stop=True)
                    ea = consts.tile([kw, 64], F16, name=f"ea_{tag}{i}")
                    nc.vector.tensor_copy(ea[:], ps[:])
                    tiles.append(ea)
                bi, br = brow
                nc.sync.dma_start(tiles[bi][br:br + 1, :], b1_ap[:])
                return tiles

            ea_a0, ea_a1 = build_ea(embT_ast_sb, astW1_sb, astb1h,
                                    [(0, 128), (128, 202)], (1, 72), "a")
            (ea_c,) = build_ea(embT_cfg_sb, cfgW1_sb, cfgb1h,
                               [(0, 102)], (0, 100), "c")

            # ---- pool PSUM accumulators; cnt*b2 is the starting matmul of
            # each accumulation position (DoubleRow outputs must start at
            # partition 0, so ast and cfg get separate [64, G] tiles) ----
            pool_ast = ps_pool.tile([64, G], F32, name="pool_ast")
            pool_cfg = ps_pool.tile([64, G], F32, name="pool_cfg")
            for (g0, g1) in ((0, 512), (512, 1024)):
                nc.tensor.matmul(pool_ast[:, g0:g1], astb2_sb[:],
                                 cnt_ast_sb[:, g0:g1], start=True, stop=False,
                                 skip_group_check=True)
                nc.tensor.matmul(pool_cfg[:, g0:g1], cfgb2_sb[:],
                                 cnt_cfg_sb[:, g0:g1], start=True, stop=False,
                                 skip_group_check=True)

            if not USE_P2P:
                # warm up the CC RDH stream so the tail ReduceScatter starts
                # hot; emitted before the T stream so the pre-CC queue drain
                # sees an empty SWDGE ring
                warm_in = dram.tile([cfg.NCORES, 1, 8], F16, name="warm_in")
                warm_out = dram.tile([1, 8], F16, name="warm_out")
                wz = small.tile([1, 64], F16, name="wz")
                nc.vector.memset(wz[:], 0.0)
                nc.sync.dma_start(
                    warm_in[:, :, :].rearrange("j p d -> p j d"),
                    wz[:].rearrange("p (j d) -> p j d", j=cfg.NCORES))
                nc.gpsimd.collective_compute(
                    "ReduceScatter", ALU.add, replica_groups=RG,
                    ins=[warm_in.opt()], outs=[warm_out.opt()])

            # ---- T granules: all emitted up-front on gpsimd (SWDGE q0);
            # tile-pool backpressure (bufs=5) paces the ring ----
            tgran = []
            for g in range(cfg.TGRAN):
                a0 = ta0_pool.tile([128, 1024], F8, name=f"ta0_{g}", tag="a0")
                nc.gpsimd.dma_start(a0[:], T_a0[g])
                a1 = ta1_pool.tile([74, 1024], F8, name=f"ta1_{g}", tag="a1")
                nc.gpsimd.dma_start(a1[:], T_a1[g])
                c_ = tcc_pool.tile([102, 1024], F8, name=f"tc_{g}", tag="c")
                nc.gpsimd.dma_start(c_[:], T_c[g])
                tgran.append((a0, a1, c_))

            # epilogue constants (issued early, vector is idle now)
            ones64 = small.tile([64, 1], F16, name="ones64")
            nc.vector.memset(ones64[:], 1.0 / 64.0)
            ones1 = small.tile([1, 64], F16, name="ones1")
            nc.vector.memset(ones1[:], 1.0)
            eps = small.tile([1, 1], F32, name="eps")
            nc.vector.memset(eps[:], 1e-5)

            # ---- C granule streams: ast on sync, cfg on scalar ----
            cgran = []

            def c_dma(q):
                ca = ca_pool.tile([128, 4096], F8, name=f"ca{q}", tag="ca")
                cc = cc_pool.tile([128, 4096], F8, name=f"cc{q}", tag="cc")
                if q < 24:
                    nc.sync.dma_start(ca[:], C_ast[q])
                    nc.scalar.dma_start(cc[:], C_cfg[q])
                else:
                    nc.sync.dma_start(ca[:, 0:2048], C_ast[24, :, 0:2048])
                    nc.scalar.dma_start(cc[:, 0:2048], C_cfg[24, :, 0:2048])
                cgran.append((ca, cc))

            for q in range(cfg.PREF_C):
                c_dma(q)
                if q == 1:
                    # sem-branch + epilogue consts, interleaved early on the
                    # two HWDGE queues (tiny vs the C stream, needed mid-kernel)
                    Wsem_sb = consts.tile([128, cfg.SEMK * 64], F16,
                                          name="Wsem_sb")
                    semT_sb = consts.tile([128, cfg.SEMK * GB], F16,
                                          name="semT_sb")
                    for kc in range(cfg.SEMK):
                        nc.sync.dma_start(Wsem_sb[:, kc * 64:(kc + 1) * 64],
                                          Wsemh[kc * 128:(kc + 1) * 128, :])
                        nc.scalar.dma_start(semT_sb[:, kc * GB:(kc + 1) * GB],
                                            semTh[kc * 128:(kc + 1) * 128, :])
                    Wg1h_sb = load_c(nc.sync, Wg1h, [128, 64], F16)
                    bg1_sb = load_c(nc.scalar, bg1c, [64, 1])
                    bsem_sb = load_c(nc.sync, bsemc, [64, 1])
                    Wg2h_sb = load_c(nc.scalar, Wg2h, [128, 64], F16)
                    bg2_sb = load_c(nc.sync, bg2c, [64, 1])
                    Wcp_sb = load_c(nc.scalar, Wcph, [64, 2], F16)
                    bcp_sb = load_c(nc.sync, bcp, [2, 1])
                    DD_sb = load_c(nc.scalar, DD, [128, 64], F16)
                    DH_sb = load_c(nc.sync, DH, [128, 64], F16)

            x2T = x2t_pool.tile([128, cfg.NBP], F16, name="x2T", tag="x2T")
            cat2 = consts.tile([128, GB], F16, name="cat2")
            pool_sb = consts.tile([128, G], F16, name="pool_sb")
            recv = consts.tile([128, G], F16, name="recv")

            def bstep(c):
                g, h = divmod(c, 2)
                sl = slice(h * 512, h * 512 + 512)
                a0, a1, c_ = tgran[g]
                px = ps_px.tile([128, 512], F32, name=f"px{c}", tag="px")
                nc.tensor.matmul(px[0:64, :], ea_a0[:], a0[:, sl],
                                 start=True, stop=False, skip_group_check=True)
                nc.tensor.matmul(px[0:64, :], ea_a1[:], a1[:, sl],
                                 start=False, stop=True, skip_group_check=True)
                nc.tensor.matmul(px[64:128, :], ea_c[:], c_[:, sl],
                                 start=True, stop=True, skip_group_check=True)
                nc.vector.tensor_scalar_max(x2T[:, c * 512:(c + 1) * 512],
                                            px[:], 0.0)

            def h2pair(s2):
                # h2 (fp8) for node blocks 2*s2, 2*s2+1; cols j*128+(0:64) hold
                # ast h2, j*128+(64:128) hold cfg h2 (block-diagonal W2)
                ph = ps_ph.tile([128, 256], F32, name=f"ph{s2}", tag="ph")
                for j in range(2):
                    blk = 2 * s2 + j
                    nc.tensor.matmul(ph[:, j * 128:(j + 1) * 128],
                                     x2T[:, blk * 128:(blk + 1) * 128],
                                     W2blk_sb[:], start=True, stop=True)
                h2q = h2p.tile([128, 256], F8, name=f"h2_{s2}", tag="h2")
                nc.vector.tensor_copy(h2q[:], ph[:])
                return h2q

            def poolstep(s2, h2q):
                stop = (s2 == 48)
                q, hh_ = divmod(s2, 2)
                ca, cc = cgran[q]
                off = hh_ * 2048
                h2v = h2q[:].rearrange("p (two t m) -> p two t m", two=2, t=2)
                for ti, (ct, pool_ps) in enumerate(((ca, pool_ast),
                                                    (cc, pool_cfg))):
                    rhs3 = ct[:, off:off + 2048].rearrange(
                        "p (two g) -> p two g", two=2)
                    lhsT = h2v[:, :, ti, :]
                    for (g0, g1) in ((0, 512), (512, 1024)):
                        nc.tensor.matmul(pool_ps[:, g0:g1], lhsT,
                                         rhs3[:, :, g0:g1],
                                         start=False, stop=stop,
                                         perf_mode=PM.DoubleRow,
                                         skip_group_check=True)

            def sem_branch():
                pssem = ps_px.tile([64, GB], F32, name="pssem", tag="px")
                for kc in range(cfg.SEMK):
                    nc.tensor.matmul(pssem[:],
                                     Wsem_sb[:, kc * 64:(kc + 1) * 64],
                                     semT_sb[:, kc * GB:(kc + 1) * GB],
                                     start=(kc == 0), stop=(kc == cfg.SEMK - 1))
                hsem = small.tile([64, GB], F16, name="hsem")
                nc.scalar.activation(hsem[:], pssem[:], AF.Relu, bias=bsem_sb[:])
                nc.gpsimd.dma_start(cat2[64:128, :], hsem[:])
                return hsem

            # ---- fused streaming loop ----
            hsem = None
            bstep(0)
            bstep(1)
            for c in range(cfg.NCHUNK):
                for s2 in (2 * c, 2 * c + 1):
                    if s2 == cfg.NPAIR - 1:
                        continue               # all-padding pair
                    h2q = h2pair(s2)
                    if s2 % 2 == 0 and c + 2 < cfg.NCHUNK:
                        bstep(c + 2)
                    poolstep(s2, h2q)
                    if s2 == cfg.SPLIT - 1:
                        hsem = sem_branch()
                if c + cfg.PREF_C < cfg.NGRAN:
                    c_dma(c + cfg.PREF_C)

            # ---- drain: evacuate pools ----
            pA = small.tile([64, G], F16, name="pA")
            nc.vector.tensor_copy(pA[:], pool_ast[:])
            pC = small.tile([64, G], F16, name="pC")
            nc.vector.tensor_copy(pC[:], pool_cfg[:])
            # warm the ACT sigmoid table while the exchange runs
            sgw = small.tile([1, 1], F16, name="sgw")
            nc.scalar.activation(sgw[:], pA[0:1, 0:1], AF.Sigmoid)

            if USE_P2P:
                nc.vector.tensor_copy(pool_sb[0:64, :], pA[:])
                nc.sync.dma_start(pool_sb[64:128, :], pC[:])
                # core m sends its position-s columns (= graph block m^s) to
                # core m^s; every core's slice at position 0 is its own block
                for d in range(1, cfg.NCORES):
                    rd = [None] * 8
                    rd[d] = (0, d)
                    nc.gpsimd.remote_dma_broadcast(
                        recv[:, d * GB:(d + 1) * GB],
                        pool_sb[:, d * GB:(d + 1) * GB],
                        remote_sem=p2p_sem, local_sem=p2p_loc,
                        rdests=rd, queue_num=3)
                # cross-core waits are invisible to the single-core scheduling
                # sim (it would flag a deadlock), so emit them with wait 0 and
                # patch the real values in after scheduling, before compile
                nc._bir_kernel_barrier_sem_replica_groups.extend(
                    set(g) for g in RG)
                w_bar = nc.gpsimd.wait_ge(nc._bir_kernel_barrier_sem, 0)
                nc.gpsimd.trigger_dma(count=None, queue_num=3)
                # 7 arrivals x (16//8)=2 increments each
                w_arr = nc.vector.wait_ge(p2p_sem, 0)
                patches.append((w_bar, nc.bir_kernel_barrier_sem_inc))
                patches.append((w_arr, 14))
                hpool = small.tile([128, GB], F16, name="hpool")
                nc.vector.tensor_add(hpool[:], pool_sb[:, 0:GB],
                                     recv[:, GB:2 * GB])
                for d in range(2, cfg.NCORES):
                    nc.vector.tensor_add(hpool[:], hpool[:],
                                         recv[:, d * GB:(d + 1) * GB])
            else:
                rs_in = dram.tile([cfg.NCORES, 128, GB], F16, name="rsin")
                rs_out = dram.tile([128, GB], F16, name="rsout")
                nc.sync.dma_start(
                    rs_in[:, 0:64, :].rearrange("j p d -> p j d"),
                    pA[:].rearrange("p (j d) -> p j d", j=cfg.NCORES))
                nc.scalar.dma_start(
                    rs_in[:, 64:128, :].rearrange("j p d -> p j d"),
                    pC[:].rearrange("p (j d) -> p j d", j=cfg.NCORES))
                nc.gpsimd.collective_compute(
                    "ReduceScatter", ALU.add, replica_groups=RG,
                    ins=[rs_in.opt()], outs=[rs_out.opt()])
                hpool = small.tile([128, GB], F16, name="hpool")
                nc.sync.dma_start(hpool[:], rs_out[:])

            # ---- epilogue for this core's GB graphs ----
            # gated fuse 1: hs = hC + g1*(hA - hC); hA-hC and hC pulled onto
            # partitions 0:64 with [I;-I] / [0;I] matmuls
            psg1 = ps_px.tile([64, GB], F32, name="psg1", tag="px")
            nc.tensor.matmul(psg1[:], Wg1h_sb[:], hpool[:], start=True, stop=True)
            psD1 = ps_ph.tile([64, GB], F32, name="psD1", tag="ph")
            nc.tensor.matmul(psD1[:], DD_sb[:], hpool[:], start=True, stop=True)
            psHC = ps_ph.tile([64, GB], F32, name="psHC", tag="ph")
            nc.tensor.matmul(psHC[:], DH_sb[:], hpool[:], start=True, stop=True)
            g1 = small.tile([64, GB], F16, name="g1")
            nc.scalar.activation(g1[:], psg1[:], AF.Sigmoid, bias=bg1_sb[:])
            t1 = small.tile([64, GB], F16, name="t1")
            nc.vector.tensor_mul(t1[:], g1[:], psD1[:])
            nc.vector.tensor_add(cat2[0:64, :], psHC[:], t1[:])

            # gated fuse 2 with the semantic branch
            psg2 = ps_px.tile([64, GB], F32, name="psg2", tag="px")
            nc.tensor.matmul(psg2[:], Wg2h_sb[:], cat2[:], start=True, stop=True)
            g2 = small.tile([64, GB], F16, name="g2")
            nc.scalar.activation(g2[:], psg2[:], AF.Sigmoid, bias=bg2_sb[:])
            d2 = small.tile([64, GB], F16, name="d2")
            nc.vector.tensor_sub(d2[:], cat2[0:64, :], hsem[:])
            t2 = small.tile([64, GB], F16, name="t2")
            nc.vector.tensor_mul(t2[:], g2[:], d2[:])
            hh = small.tile([64, 2 * GB], F16, name="hh")
            nc.vector.tensor_add(hh[:, 0:GB], hsem[:], t2[:])
            nc.vector.tensor_mul(hh[:, GB:2 * GB], hh[:, 0:GB], hh[:, 0:GB])

            # LayerNorm folded into classifier: out = ((h-mu)*rstd) @ Wc' + bc'
            ps2 = ps_ph.tile([1, 2 * GB], F32, name="ps2", tag="ph")
            nc.tensor.matmul(ps2[:], ones64[:], hh[:], start=True, stop=True)
            row2 = small.tile([1, 2 * GB], F32, name="row2")
            nc.vector.tensor_copy(row2[:], ps2[:])
            mu2 = small.tile([1, GB], F32, name="mu2")
            nc.vector.tensor_mul(mu2[:], row2[:, 0:GB], row2[:, 0:GB])
            var = small.tile([1, GB], F32, name="var")
            nc.vector.tensor_sub(var[:], row2[:, GB:2 * GB], mu2[:])
            brow = small.tile([1, 2 * GB], F16, name="brow")
            nc.scalar.activation(brow[:, 0:GB], var[:], AF.Abs_reciprocal_sqrt,
                                 bias=eps[:])
            nc.vector.tensor_mul(brow[:, GB:2 * GB], row2[:, 0:GB],
                                 brow[:, 0:GB])
            psb = ps_px.tile([64, 2 * GB], F32, name="psb", tag="px")
            nc.tensor.matmul(psb[:], ones1[:], brow[:], start=True, stop=True)
            z = small.tile([64, GB], F16, name="z")
            nc.vector.tensor_mul(z[:], hh[:, 0:GB], psb[:, 0:GB])
            nc.vector.tensor_sub(z[:], z[:], psb[:, GB:2 * GB])
            psout = ps_ph.tile([2, GB], F32, name="psout", tag="ph")
            nc.tensor.matmul(psout[:], Wcp_sb[:], z[:], start=True, stop=True)
            outT_sb = small.tile([2, GB], F32, name="outT_sb")
            nc.vector.tensor_scalar_add(outT_sb[:], psout[:], bcp_sb[:])
            nc.sync.dma_start(out_ap[:], outT_sb[:])

    for inst, val in patches:
        inst.ins.sync_info.on_wait[0].wait_value = val

    nc.compile()
    return nc


# ---------------------------------------------------------------------------
# host-side preprocessing
# ---------------------------------------------------------------------------

def preprocess(inputs: dict, cfg: CFG):
    N, G, NB, NBP, NBPT, GB = cfg.N, cfg.G, cfg.NB, cfg.NBP, cfg.NBPT, cfg.GB

    def graph_structs(edge, types, batch, nt2, nt):
        src = np.asarray(edge[0], np.int64)
        dst = np.asarray(edge[1], np.int64)
        types = np.asarray(types, np.int64)
        batch = np.asarray(batch, np.int64)
        deg = (np.bincount(dst, minlength=N) + 1.0).astype(np.float32)
        dinv = (1.0 / np.sqrt(deg)).astype(np.float32)
        coeff = (dinv[src] * dinv[dst]).astype(np.float32)
        selfc = (dinv * dinv).astype(np.float32)
        t_src = types[src]
        g_dst = batch[dst]
        counts = np.bincount(batch, minlength=G).astype(np.float32)
        Ts, Cs, cnts = [], [], []
        for k in range(cfg.NCORES):
            lo, hi = k * NB, (k + 1) * NB
            m = (dst >= lo) & (dst < hi)
            flat = t_src[m] * NBPT + (dst[m] - lo)
            T = np.bincount(flat, weights=coeff[m].astype(np.float64),
                            minlength=nt2 * NBPT)
            blk = np.arange(lo, hi)
            flat_self = types[blk] * NBPT + (blk - lo)
            T += np.bincount(flat_self, weights=selfc[blk].astype(np.float64),
                             minlength=nt2 * NBPT)
            T = T.reshape(nt2, NBPT).astype(np.float32)
            T[nt, 0:NB] = 1.0   # bias row
            Ts.append(T.astype(NP_F8))
            m2 = (src >= lo) & (src < hi)
            gd = g_dst[m2]
            if USE_P2P:
                gd = ((gd >> 7) ^ k) * 128 + (gd & 127)
            flat2 = (src[m2] - lo) * G + gd
            C = np.bincount(flat2, weights=coeff[m2].astype(np.float64),
                            minlength=NBP * G)
            gs = batch[blk]
            if USE_P2P:
                gs = ((gs >> 7) ^ k) * 128 + (gs & 127)
            flat2s = (blk - lo) * G + gs
            C += np.bincount(flat2s, weights=selfc[blk].astype(np.float64),
                             minlength=NBP * G)
            C = C.reshape(NBP, G).astype(np.float32).astype(NP_F8)
            C49 = C[0:12544].reshape(49, 2, 128, G).transpose(0, 2, 1, 3) \
                .reshape(49, 128, 2048)
            C50 = np.zeros((50, 128, 2048), NP_F8)
            C50[0:49] = C49
            Cg = np.ascontiguousarray(
                C50.reshape(25, 2, 128, 2048).transpose(0, 2, 1, 3)
                .reshape(25, 128, 4096))
            Cs.append(Cg)
            cm = np.zeros((1, G), np.float16)
            if USE_P2P:
                cm[0, 0:GB] = counts[k * GB:(k + 1) * GB]
            else:
                cm[0, k * GB:(k + 1) * GB] = counts[k * GB:(k + 1) * GB]
            cnts.append(cm)
        return Ts, Cs, cnts

    Ta, Ca, cnta = graph_structs(inputs["ast_edge"], inputs["ast_type"],
                                 inputs["ast_batch"], cfg.NTA2, cfg.NTA)
    Tc, Cc, cntc = graph_structs(inputs["cfg_edge"], inputs["cfg_type"],
                                 inputs["cfg_batch"], cfg.NTC2, cfg.NTC)

    f32 = lambda x: np.ascontiguousarray(np.asarray(x, np.float32))
    f16 = lambda x: np.ascontiguousarray(np.asarray(x, np.float32).astype(np.float16))
    embT_ast = np.zeros((64, cfg.NTA2), np.float32)
    embT_ast[:, 0:cfg.NTA] = f32(inputs["ast_emb"]).T
    embT_cfg = np.zeros((64, cfg.NTC2), np.float32)
    embT_cfg[:, 0:cfg.NTC] = f32(inputs["cfg_emb"]).T
    semT = f32(inputs["struct_sem"]).T.copy()  # [SEM, G]

    ln_g = f32(inputs["ln_g"])
    ln_b = f32(inputs["ln_b"])
    Wc = f32(inputs["Wc"])
    Wcph = (ln_g[:, None] * Wc).astype(np.float16)
    W2blk = np.zeros((128, 128), np.float16)
    W2blk[0:64, 0:64] = f16(inputs["ast_W2"])
    W2blk[64:128, 64:128] = f16(inputs["cfg_W2"])
    bcp = (ln_b @ Wc + f32(inputs["bc"])).reshape(2, 1)
    DDm = np.zeros((128, 64), np.float16)
    DHm = np.zeros((128, 64), np.float16)
    DDm[0:64] = np.eye(64, dtype=np.float16)
    DDm[64:128] = -np.eye(64, dtype=np.float16)
    DHm[64:128] = np.eye(64, dtype=np.float16)

    shared = {
        "embT_ast": embT_ast, "embT_cfg": embT_cfg,
        "astW1": f32(inputs["ast_W1"]), "cfgW1": f32(inputs["cfg_W1"]),
        "astb1h": f16(inputs["ast_b1"]).reshape(1, 64),
        "cfgb1h": f16(inputs["cfg_b1"]).reshape(1, 64),
        "W2blk": W2blk,
        "astb2h": f16(inputs["ast_b2"]).reshape(1, 64),
        "cfgb2h": f16(inputs["cfg_b2"]).reshape(1, 64),
        "Wg1h": f16(inputs["Wg1"]), "bg1c": f32(inputs["bg1"]).reshape(64, 1),
        "Wsemh": f16(inputs["Wsem"]), "bsemc": f32(inputs["bsem"]).reshape(64, 1),
        "Wg2h": f16(inputs["Wg2"]), "bg2c": f32(inputs["bg2"]).reshape(64, 1),
        "Wcph": np.ascontiguousarray(Wcph), "bcp": np.ascontiguousarray(bcp),
        "DD": DDm, "DH": DHm,
    }

    def pack_T(T, nt2):
        # [nt2, NBPT] -> granule layout [13, rows, 1024]
        Tg = np.ascontiguousarray(
            T.reshape(nt2, cfg.TGRAN, 1024).transpose(1, 0, 2))
        return Tg

    in_maps = []
    for k in range(cfg.NCORES):
        m = dict(shared)
        Tg_a = pack_T(Ta[k], cfg.NTA2)
        m["T_a0"] = np.ascontiguousarray(Tg_a[:, 0:128])
        m["T_a1"] = np.ascontiguousarray(Tg_a[:, 128:202])
        m["T_c"] = pack_T(Tc[k], cfg.NTC2)[:, 0:102]
        m["T_c"] = np.ascontiguousarray(m["T_c"])
        m["C_ast"] = Ca[k]
        m["C_cfg"] = Cc[k]
        m["cnt_ast"] = cnta[k]
        m["cnt_cfg"] = cntc[k]
        m["semTh"] = np.ascontiguousarray(
            semT[:, k * GB:(k + 1) * GB].astype(np.float16))
        in_maps.append(m)
    return in_maps


def postprocess(results, cfg: CFG):
    outs = [np.asarray(results[k]["outT"]) for k in range(cfg.NCORES)]
    return np.concatenate(outs, axis=1).T.copy()  # [G, 2]


_CACHED = {}


def kernel(**inputs):
    from concourse.bass_utils import run_bass_kernel_spmd
    cfg = CFG()
    if "nc" not in _CACHED:
        _CACHED["nc"] = build_nc(cfg)
    in_maps = preprocess(inputs, cfg)
    res = run_bass_kernel_spmd(_CACHED["nc"], in_maps,
                               core_ids=list(range(cfg.NCORES)))
    return postprocess(res.results, cfg)


# revision 33
# speedup vs baseline: 1.1243x; 1.1243x over previous
"""Trainium2 Bass kernel for nn_CrossGraphNetLite (dual-GNN + gated fusion + classifier).

Strategy (8 NeuronCores, graph/data parallel, fp8 streams):
  * Host preprocesses the integer graph structure into dense coefficient
    matrices, quantized to fp8 e4m3:
      - Layer 1 per dst-node block:  T[t, v] = sum of edge coeffs into v
        bucketed by source-node *type* t (+ self-loop + bias row). On device
        x2 = relu(ea^T T) with ea = [emb @ W1; b1] in fp16.
      - Layer 2 + mean-pool collapse: C[s, g] = sum of edge coeffs from src s
        (this core's block) into any node of graph g (+ self-loop), fp8.
        pool^T += h2[pair]^T C[pair] in DoubleRow fp8 perf mode.
  * C tiles stream on the sync+scalar HWDGE queues as [128, 4096] granules
    (2 KB contiguous per-partition descriptors); T streams on gpsimd SWDGE.
  * The cross-core reduction avoids the ~26 us RDH ReduceScatter: C's graph
    columns are XOR-permuted per core (graph block b sits at position b^core)
    so identical SPMD remote_dma_broadcast instructions with *relative* dests
    deliver exactly the receiver's 128 graphs: core m sends its position-s
    slice with delta s to core m^s. Receivers sum 8 slices locally.
  * Tight epilogue: hA-hC / hC extracted via [I;-I],[0;I] identity matmuls
    (no cross-partition DVE), LayerNorm folded into the classifier weights.
"""

import sys

sys.path.insert(0, "/opt/trn_rl_repo")

import numpy as np
import ml_dtypes

import concourse.bacc as bacc
import concourse.bass as bass
import concourse.mybir as mybir
import concourse.tile as tile

AF = mybir.ActivationFunctionType
ALU = mybir.AluOpType
PM = mybir.MatmulPerfMode
F32 = mybir.dt.float32
F16 = mybir.dt.float16
F8 = mybir.dt.float8e4
NP_F8 = ml_dtypes.float8_e4m3

USE_P2P = False


class CFG:
    def __init__(self):
        self.N = 100000
        self.E = 1250000
        self.G = 1024
        self.NCORES = 8
        self.NTA = 200                             # + bias row -> 201, pad 202
        self.NTC = 100                             # + bias row -> 101, pad 102
        self.NTA2 = 202
        self.NTC2 = 102
        self.SEM = 768
        self.NB = self.N // self.NCORES            # 12500
        self.NBP = 12800                           # 25 x 512 (x2T width)
        self.NBPT = 13312                          # 13 x 1024 (T storage)
        self.NCHUNK = 25                           # 512-node chunks
        self.NPAIR = 50                            # 256-node pairs (49 real)
        self.NGRAN = 25                            # C granules (2 pairs each)
        self.TGRAN = 13                            # T granules (2 chunks each)
        self.GB = self.G // self.NCORES            # 128
        self.SEMK = 6
        self.SPLIT = 35                            # sem branch after pair 34
        self.PREF_C = 8                            # C granule prefetch depth


def build_nc(cfg: CFG):
    nc = bacc.Bacc("TRN2", target_bir_lowering=False, debug=False,
                   enable_asserts=False, num_devices=cfg.NCORES,
                   num_swdge_queues=4)
    G, GB = cfg.G, cfg.GB
    RG = [list(range(cfg.NCORES))]

    def din(name, shape, dt=F32):
        return nc.dram_tensor(name, list(shape), dt, kind="ExternalInput").ap()

    T_a0 = din("T_a0", [cfg.TGRAN, 128, 1024], F8)
    T_a1 = din("T_a1", [cfg.TGRAN, 74, 1024], F8)
    T_c = din("T_c", [cfg.TGRAN, 102, 1024], F8)
    # granule-packed: [q, p, 4096] = pairs (2q, 2q+1) contiguous per
    # partition -> one 4 KB DMA descriptor per partition
    C_ast = din("C_ast", [cfg.NGRAN, 128, 4096], F8)
    C_cfg = din("C_cfg", [cfg.NGRAN, 128, 4096], F8)
    embT_ast = din("embT_ast", [64, cfg.NTA2])
    embT_cfg = din("embT_cfg", [64, cfg.NTC2])
    astW1 = din("astW1", [64, 64])
    cfgW1 = din("cfgW1", [64, 64])
    astb1h = din("astb1h", [1, 64], F16)
    cfgb1h = din("cfgb1h", [1, 64], F16)
    W2blk = din("W2blk", [128, 128], F16)
    astb2h = din("astb2h", [1, 64], F16)
    cfgb2h = din("cfgb2h", [1, 64], F16)
    cnt_ast = din("cnt_ast", [1, G], F16)
    cnt_cfg = din("cnt_cfg", [1, G], F16)
    Wg1h = din("Wg1h", [128, 64], F16)
    bg1c = din("bg1c", [64, 1])
    Wsemh = din("Wsemh", [cfg.SEM, 64], F16)
    bsemc = din("bsemc", [64, 1])
    semTh = din("semTh", [cfg.SEM, GB], F16)
    Wg2h = din("Wg2h", [128, 64], F16)
    bg2c = din("bg2c", [64, 1])
    Wcph = din("Wcph", [64, 2], F16)
    bcp = din("bcp", [2, 1])
    DD = din("DD", [128, 64], F16)                 # [I; -I]
    DH = din("DH", [128, 64], F16)                 # [0; I]
    out_ap = nc.dram_tensor("outT", [2, GB], F32, kind="ExternalOutput").ap()

    if USE_P2P:
        p2p_sem = nc.alloc_semaphore("p2p_arrival")
        p2p_loc = nc.alloc_semaphore("p2p_local")
    patches = []

    with tile.TileContext(nc) as tc:
        with (
            tc.tile_pool(name="consts", bufs=1) as consts,
            tc.tile_pool(name="x2t", bufs=1) as x2t_pool,
            tc.tile_pool(name="ta0", bufs=5) as ta0_pool,
            tc.tile_pool(name="ta1", bufs=5) as ta1_pool,
            tc.tile_pool(name="tcc", bufs=5) as tcc_pool,
            tc.tile_pool(name="ca", bufs=10) as ca_pool,
            tc.tile_pool(name="cc", bufs=10) as cc_pool,
            tc.tile_pool(name="h2p", bufs=8) as h2p,
            tc.tile_pool(name="small", bufs=1) as small,
            tc.tile_pool(name="ps_px", bufs=2, space="PSUM") as ps_px,
            tc.tile_pool(name="ps_ph", bufs=2, space="PSUM") as ps_ph,
            tc.tile_pool(name="ps_pool", bufs=1, space="PSUM") as ps_pool,
            tc.tile_pool(name="dram", bufs=1, space="DRAM") as dram,
        ):
            # ---- critical consts: sync feeds the ea build, scalar feeds
            # W2/cnt/b2 (needed within a few us) ----
            def load_c(eng, ap, shape, dt=F32, name=None):
                t = consts.tile(list(shape), dt, name=name or ap.tensor.name + "_sb")
                eng.dma_start(t[:], ap[:])
                return t

            embT_ast_sb = load_c(nc.sync, embT_ast, [64, cfg.NTA2])
            embT_cfg_sb = load_c(nc.sync, embT_cfg, [64, cfg.NTC2])
            astW1_sb = load_c(nc.sync, astW1, [64, 64])
            cfgW1_sb = load_c(nc.sync, cfgW1, [64, 64])
            W2blk_sb = load_c(nc.scalar, W2blk, [128, 128], F16)
            astb2_sb = load_c(nc.scalar, astb2h, [1, 64], F16)
            cfgb2_sb = load_c(nc.scalar, cfgb2h, [1, 64], F16)
            cnt_ast_sb = load_c(nc.scalar, cnt_ast, [1, G], F16)
            cnt_cfg_sb = load_c(nc.scalar, cnt_cfg, [1, G], F16)

            # ---- ea tables: [emb @ W1 ; b1] in fp16 ----
            def build_ea(embT_sb, W1_sb, b1_ap, ksl, brow, tag):
                tiles = []
                for i, (k0, k1) in enumerate(ksl):
                    kw = k1 - k0
                    ps = ps_ph.tile([kw, 64], F32, name=f"psea_{tag}{i}", tag="ph")
                    nc.tensor.matmul(ps[:], embT_sb[:, k0:k1], W1_sb[:],
                                     start=True, stop=True)
                    ea = consts.tile([kw, 64], F16, name=f"ea_{tag}{i}")
                    nc.vector.tensor_copy(ea[:], ps[:])
                    tiles.append(ea)
                bi, br = brow
                nc.sync.dma_start(tiles[bi][br:br + 1, :], b1_ap[:])
                return tiles

            ea_a0, ea_a1 = build_ea(embT_ast_sb, astW1_sb, astb1h,
                                    [(0, 128), (128, 202)], (1, 72), "a")
            (ea_c,) = build_ea(embT_cfg_sb, cfgW1_sb, cfgb1h,
                               [(0, 102)], (0, 100), "c")

            # ---- pool PSUM accumulators; cnt*b2 is the starting matmul of
            # each accumulation position (DoubleRow outputs must start at
            # partition 0, so ast and cfg get separate [64, G] tiles) ----
            pool_ast = ps_pool.tile([64, G], F32, name="pool_ast")
            pool_cfg = ps_pool.tile([64, G], F32, name="pool_cfg")
            for (g0, g1) in ((0, 512), (512, 1024)):
                nc.tensor.matmul(pool_ast[:, g0:g1], astb2_sb[:],
                                 cnt_ast_sb[:, g0:g1], start=True, stop=False,
                                 skip_group_check=True)
                nc.tensor.matmul(pool_cfg[:, g0:g1], cfgb2_sb[:],
                                 cnt_cfg_sb[:, g0:g1], start=True, stop=False,
                                 skip_group_check=True)

            # ---- T granules: all emitted up-front on gpsimd (SWDGE q0);
            # tile-pool backpressure (bufs=5) paces the ring ----
            tgran = []
            for g in range(cfg.TGRAN):
                a0 = ta0_pool.tile([128, 1024], F8, name=f"ta0_{g}", tag="a0")
                nc.gpsimd.dma_start(a0[:], T_a0[g])
                a1 = ta1_pool.tile([74, 1024], F8, name=f"ta1_{g}", tag="a1")
                nc.gpsimd.dma_start(a1[:], T_a1[g])
                c_ = tcc_pool.tile([102, 1024], F8, name=f"tc_{g}", tag="c")
                nc.gpsimd.dma_start(c_[:], T_c[g])
                tgran.append((a0, a1, c_))

            if not USE_P2P:
                # warm up the CC RDH stream so the tail ReduceScatter starts hot
                warm_in = dram.tile([cfg.NCORES, 1, 8], F16, name="warm_in")
                warm_out = dram.tile([1, 8], F16, name="warm_out")
                wz = small.tile([1, 64], F16, name="wz")
                nc.vector.memset(wz[:], 0.0)
                nc.sync.dma_start(
                    warm_in[:, :, :].rearrange("j p d -> p j d"),
                    wz[:].rearrange("p (j d) -> p j d", j=cfg.NCORES))
                nc.gpsimd.collective_compute(
                    "ReduceScatter", ALU.add, replica_groups=RG,
                    ins=[warm_in.opt()], outs=[warm_out.opt()])

            # epilogue constants (issued early, vector is idle now)
            ones64 = small.tile([64, 1], F16, name="ones64")
            nc.vector.memset(ones64[:], 1.0 / 64.0)
            ones1 = small.tile([1, 64], F16, name="ones1")
            nc.vector.memset(ones1[:], 1.0)
            eps = small.tile([1, 1], F32, name="eps")
            nc.vector.memset(eps[:], 1e-5)

            # ---- C granule streams: ast on sync, cfg on scalar ----
            cgran = []

            def c_dma(q):
                ca = ca_pool.tile([128, 4096], F8, name=f"ca{q}", tag="ca")
                cc = cc_pool.tile([128, 4096], F8, name=f"cc{q}", tag="cc")
                if q < 24:
                    nc.sync.dma_start(ca[:], C_ast[q])
                    nc.scalar.dma_start(cc[:], C_cfg[q])
                else:
                    nc.sync.dma_start(ca[:, 0:2048], C_ast[24, :, 0:2048])
                    nc.scalar.dma_start(cc[:, 0:2048], C_cfg[24, :, 0:2048])
                cgran.append((ca, cc))

            for q in range(cfg.PREF_C):
                c_dma(q)
                if q == 1:
                    # sem-branch + epilogue consts, interleaved early on the
                    # two HWDGE queues (tiny vs the C stream, needed mid-kernel)
                    Wsem_sb = consts.tile([128, cfg.SEMK * 64], F16,
                                          name="Wsem_sb")
                    semT_sb = consts.tile([128, cfg.SEMK * GB], F16,
                                          name="semT_sb")
                    for kc in range(cfg.SEMK):
                        nc.sync.dma_start(Wsem_sb[:, kc * 64:(kc + 1) * 64],
                                          Wsemh[kc * 128:(kc + 1) * 128, :])
                        nc.scalar.dma_start(semT_sb[:, kc * GB:(kc + 1) * GB],
                                            semTh[kc * 128:(kc + 1) * 128, :])
                    Wg1h_sb = load_c(nc.sync, Wg1h, [128, 64], F16)
                    bg1_sb = load_c(nc.scalar, bg1c, [64, 1])
                    bsem_sb = load_c(nc.sync, bsemc, [64, 1])
                    Wg2h_sb = load_c(nc.scalar, Wg2h, [128, 64], F16)
                    bg2_sb = load_c(nc.sync, bg2c, [64, 1])
                    Wcp_sb = load_c(nc.scalar, Wcph, [64, 2], F16)
                    bcp_sb = load_c(nc.sync, bcp, [2, 1])
                    DD_sb = load_c(nc.scalar, DD, [128, 64], F16)
                    DH_sb = load_c(nc.sync, DH, [128, 64], F16)

            x2T = x2t_pool.tile([128, cfg.NBP], F16, name="x2T", tag="x2T")
            cat2 = consts.tile([128, GB], F16, name="cat2")
            pool_sb = consts.tile([128, G], F16, name="pool_sb")
            recv = consts.tile([128, G], F16, name="recv")

            def bstep(c):
                g, h = divmod(c, 2)
                sl = slice(h * 512, h * 512 + 512)
                a0, a1, c_ = tgran[g]
                px = ps_px.tile([128, 512], F32, name=f"px{c}", tag="px")
                nc.tensor.matmul(px[0:64, :], ea_a0[:], a0[:, sl],
                                 start=True, stop=False, skip_group_check=True)
                nc.tensor.matmul(px[0:64, :], ea_a1[:], a1[:, sl],
                                 start=False, stop=True, skip_group_check=True)
                nc.tensor.matmul(px[64:128, :], ea_c[:], c_[:, sl],
                                 start=True, stop=True, skip_group_check=True)
                nc.vector.tensor_scalar_max(x2T[:, c * 512:(c + 1) * 512],
                                            px[:], 0.0)

            def h2pair(s2):
                # h2 (fp8) for node blocks 2*s2, 2*s2+1; cols j*128+(0:64) hold
                # ast h2, j*128+(64:128) hold cfg h2 (block-diagonal W2)
                ph = ps_ph.tile([128, 256], F32, name=f"ph{s2}", tag="ph")
                for j in range(2):
                    blk = 2 * s2 + j
                    nc.tensor.matmul(ph[:, j * 128:(j + 1) * 128],
                                     x2T[:, blk * 128:(blk + 1) * 128],
                                     W2blk_sb[:], start=True, stop=True)
                h2q = h2p.tile([128, 256], F8, name=f"h2_{s2}", tag="h2")
                nc.vector.tensor_copy(h2q[:], ph[:])
                return h2q

            def poolstep(s2, h2q):
                stop = (s2 == 48)
                q, hh_ = divmod(s2, 2)
                ca, cc = cgran[q]
                off = hh_ * 2048
                h2v = h2q[:].rearrange("p (two t m) -> p two t m", two=2, t=2)
                for ti, (ct, pool_ps) in enumerate(((ca, pool_ast),
                                                    (cc, pool_cfg))):
                    rhs3 = ct[:, off:off + 2048].rearrange(
                        "p (two g) -> p two g", two=2)
                    lhsT = h2v[:, :, ti, :]
                    for (g0, g1) in ((0, 512), (512, 1024)):
                        nc.tensor.matmul(pool_ps[:, g0:g1], lhsT,
                                         rhs3[:, :, g0:g1],
                                         start=False, stop=stop,
                                         perf_mode=PM.DoubleRow,
                                         skip_group_check=True)

            def sem_branch():
                pssem = ps_px.tile([64, GB], F32, name="pssem", tag="px")
                for kc in range(cfg.SEMK):
                    nc.tensor.matmul(pssem[:],
                                     Wsem_sb[:, kc * 64:(kc + 1) * 64],
                                     semT_sb[:, kc * GB:(kc + 1) * GB],
                                     start=(kc == 0), stop=(kc == cfg.SEMK - 1))
                hsem = small.tile([64, GB], F16, name="hsem")
                nc.scalar.activation(hsem[:], pssem[:], AF.Relu, bias=bsem_sb[:])
                nc.gpsimd.dma_start(cat2[64:128, :], hsem[:])
                return hsem

            # ---- fused streaming loop ----
            hsem = None
            bstep(0)
            bstep(1)
            for c in range(cfg.NCHUNK):
                for s2 in (2 * c, 2 * c + 1):
                    if s2 == cfg.NPAIR - 1:
                        continue               # all-padding pair
                    h2q = h2pair(s2)
                    if s2 % 2 == 0 and c + 2 < cfg.NCHUNK:
                        bstep(c + 2)
                    poolstep(s2, h2q)
                    if s2 == cfg.SPLIT - 1:
                        hsem = sem_branch()
                if c + cfg.PREF_C < cfg.NGRAN:
                    c_dma(c + cfg.PREF_C)

            # ---- drain: evacuate pools ----
            pA = small.tile([64, G], F16, name="pA")
            nc.vector.tensor_copy(pA[:], pool_ast[:])
            pC = small.tile([64, G], F16, name="pC")
            nc.vector.tensor_copy(pC[:], pool_cfg[:])
            # warm the ACT sigmoid table while the exchange runs
            sgw = small.tile([1, 1], F16, name="sgw")
            nc.scalar.activation(sgw[:], pA[0:1, 0:1], AF.Sigmoid)

            if USE_P2P:
                nc.vector.tensor_copy(pool_sb[0:64, :], pA[:])
                # SBUF->SBUF partition move must ride the SWDGE (gpsimd)
                # queue -- HWDGE SBUF->SBUF wedges the device
                nc.gpsimd.dma_start(pool_sb[64:128, :], pC[:])
                # core m sends its position-s columns (= graph block m^s) to
                # core m^s; every core's slice at position 0 is its own block
                for d in range(1, cfg.NCORES):
                    rd = [None] * 8
                    rd[d] = (0, d)
                    nc.gpsimd.remote_dma_broadcast(
                        recv[:, d * GB:(d + 1) * GB],
                        pool_sb[:, d * GB:(d + 1) * GB],
                        remote_sem=p2p_sem, local_sem=p2p_loc,
                        rdests=rd, queue_num=3)
                # cross-core waits are invisible to the single-core scheduling
                # sim (it would flag a deadlock), so emit them with wait 0 and
                # patch the real values in after scheduling, before compile
                nc._bir_kernel_barrier_sem_replica_groups.extend(
                    set(g) for g in RG)
                w_bar = nc.gpsimd.wait_ge(nc._bir_kernel_barrier_sem, 0)
                nc.gpsimd.trigger_dma(count=None, queue_num=3)
                # 7 arrivals x (16//8)=2 increments each
                w_arr = nc.vector.wait_ge(p2p_sem, 0)
                patches.append((w_bar, nc.bir_kernel_barrier_sem_inc))
                patches.append((w_arr, 14))
                hpool = small.tile([128, GB], F16, name="hpool")
                nc.vector.tensor_add(hpool[:], pool_sb[:, 0:GB],
                                     recv[:, GB:2 * GB])
                for d in range(2, cfg.NCORES):
                    nc.vector.tensor_add(hpool[:], hpool[:],
                                         recv[:, d * GB:(d + 1) * GB])
            else:
                rs_in = dram.tile([cfg.NCORES, 128, GB], F16, name="rsin")
                rs_out = dram.tile([128, GB], F16, name="rsout")
                nc.sync.dma_start(
                    rs_in[:, 0:64, :].rearrange("j p d -> p j d"),
                    pA[:].rearrange("p (j d) -> p j d", j=cfg.NCORES))
                nc.scalar.dma_start(
                    rs_in[:, 64:128, :].rearrange("j p d -> p j d"),
                    pC[:].rearrange("p (j d) -> p j d", j=cfg.NCORES))
                nc.gpsimd.collective_compute(
                    "ReduceScatter", ALU.add, replica_groups=RG,
                    ins=[rs_in.opt()], outs=[rs_out.opt()])
                hpool = small.tile([128, GB], F16, name="hpool")
                nc.sync.dma_start(hpool[:], rs_out[:])

            # ---- epilogue for this core's GB graphs ----
            # gated fuse 1: hs = hC + g1*(hA - hC); hA-hC and hC pulled onto
            # partitions 0:64 with [I;-I] / [0;I] matmuls
            psg1 = ps_px.tile([64, GB], F32, name="psg1", tag="px")
            nc.tensor.matmul(psg1[:], Wg1h_sb[:], hpool[:], start=True, stop=True)
            psD1 = ps_ph.tile([64, GB], F32, name="psD1", tag="ph")
            nc.tensor.matmul(psD1[:], DD_sb[:], hpool[:], start=True, stop=True)
            psHC = ps_ph.tile([64, GB], F32, name="psHC", tag="ph")
            nc.tensor.matmul(psHC[:], DH_sb[:], hpool[:], start=True, stop=True)
            g1 = small.tile([64, GB], F16, name="g1")
            nc.scalar.activation(g1[:], psg1[:], AF.Sigmoid, bias=bg1_sb[:])
            t1 = small.tile([64, GB], F16, name="t1")
            nc.vector.tensor_mul(t1[:], g1[:], psD1[:])
            nc.vector.tensor_add(cat2[0:64, :], psHC[:], t1[:])

            # gated fuse 2 with the semantic branch
            psg2 = ps_px.tile([64, GB], F32, name="psg2", tag="px")
            nc.tensor.matmul(psg2[:], Wg2h_sb[:], cat2[:], start=True, stop=True)
            g2 = small.tile([64, GB], F16, name="g2")
            nc.scalar.activation(g2[:], psg2[:], AF.Sigmoid, bias=bg2_sb[:])
            d2 = small.tile([64, GB], F16, name="d2")
            nc.vector.tensor_sub(d2[:], cat2[0:64, :], hsem[:])
            t2 = small.tile([64, GB], F16, name="t2")
            nc.vector.tensor_mul(t2[:], g2[:], d2[:])
            hh = small.tile([64, 2 * GB], F16, name="hh")
            nc.vector.tensor_add(hh[:, 0:GB], hsem[:], t2[:])
            nc.vector.tensor_mul(hh[:, GB:2 * GB], hh[:, 0:GB], hh[:, 0:GB])

            # LayerNorm folded into classifier: out = ((h-mu)*rstd) @ Wc' + bc'
            ps2 = ps_ph.tile([1, 2 * GB], F32, name="ps2", tag="ph")
            nc.tensor.matmul(ps2[:], ones64[:], hh[:], start=True, stop=True)
            row2 = small.tile([1, 2 * GB], F32, name="row2")
            nc.vector.tensor_copy(row2[:], ps2[:])
            mu2 = small.tile([1, GB], F32, name="mu2")
            nc.vector.tensor_mul(mu2[:], row2[:, 0:GB], row2[:, 0:GB])
            var = small.tile([1, GB], F32, name="var")
            nc.vector.tensor_sub(var[:], row2[:, GB:2 * GB], mu2[:])
            brow = small.tile([1, 2 * GB], F16, name="brow")
            nc.scalar.activation(brow[:, 0:GB], var[:], AF.Abs_reciprocal_sqrt,
                                 bias=eps[:])
            nc.vector.tensor_mul(brow[:, GB:2 * GB], row2[:, 0:GB],
                                 brow[:, 0:GB])
            psb = ps_px.tile([64, 2 * GB], F32, name="psb", tag="px")
            nc.tensor.matmul(psb[:], ones1[:], brow[:], start=True, stop=True)
            z = small.tile([64, GB], F16, name="z")
            nc.vector.tensor_mul(z[:], hh[:, 0:GB], psb[:, 0:GB])
            nc.vector.tensor_sub(z[:], z[:], psb[:, GB:2 * GB])
            psout = ps_ph.tile([2, GB], F32, name="psout", tag="ph")
            nc.tensor.matmul(psout[:], Wcp_sb[:], z[:], start=True, stop=True)
            outT_sb = small.tile([2, GB], F32, name="outT_sb")
            nc.vector.tensor_scalar_add(outT_sb[:], psout[:], bcp_sb[:])
            nc.sync.dma_start(out_ap[:], outT_sb[:])

    for inst, val in patches:
        inst.ins.sync_info.on_wait[0].wait_value = val

    nc.compile()
    return nc


# ---------------------------------------------------------------------------
# host-side preprocessing
# ---------------------------------------------------------------------------

def preprocess(inputs: dict, cfg: CFG):
    N, G, NB, NBP, NBPT, GB = cfg.N, cfg.G, cfg.NB, cfg.NBP, cfg.NBPT, cfg.GB

    def graph_structs(edge, types, batch, nt2, nt):
        src = np.asarray(edge[0], np.int64)
        dst = np.asarray(edge[1], np.int64)
        types = np.asarray(types, np.int64)
        batch = np.asarray(batch, np.int64)
        deg = (np.bincount(dst, minlength=N) + 1.0).astype(np.float32)
        dinv = (1.0 / np.sqrt(deg)).astype(np.float32)
        coeff = (dinv[src] * dinv[dst]).astype(np.float32)
        selfc = (dinv * dinv).astype(np.float32)
        t_src = types[src]
        g_dst = batch[dst]
        counts = np.bincount(batch, minlength=G).astype(np.float32)
        Ts, Cs, cnts = [], [], []
        for k in range(cfg.NCORES):
            lo, hi = k * NB, (k + 1) * NB
            m = (dst >= lo) & (dst < hi)
            flat = t_src[m] * NBPT + (dst[m] - lo)
            T = np.bincount(flat, weights=coeff[m].astype(np.float64),
                            minlength=nt2 * NBPT)
            blk = np.arange(lo, hi)
            flat_self = types[blk] * NBPT + (blk - lo)
            T += np.bincount(flat_self, weights=selfc[blk].astype(np.float64),
                             minlength=nt2 * NBPT)
            T = T.reshape(nt2, NBPT).astype(np.float32)
            T[nt, 0:NB] = 1.0   # bias row
            Ts.append(T.astype(NP_F8))
            m2 = (src >= lo) & (src < hi)
            gd = g_dst[m2]
            if USE_P2P:
                gd = ((gd >> 7) ^ k) * 128 + (gd & 127)
            flat2 = (src[m2] - lo) * G + gd
            C = np.bincount(flat2, weights=coeff[m2].astype(np.float64),
                            minlength=NBP * G)
            gs = batch[blk]
            if USE_P2P:
                gs = ((gs >> 7) ^ k) * 128 + (gs & 127)
            flat2s = (blk - lo) * G + gs
            C += np.bincount(flat2s, weights=selfc[blk].astype(np.float64),
                             minlength=NBP * G)
            C = C.reshape(NBP, G).astype(np.float32).astype(NP_F8)
            C49 = C[0:12544].reshape(49, 2, 128, G).transpose(0, 2, 1, 3) \
                .reshape(49, 128, 2048)
            C50 = np.zeros((50, 128, 2048), NP_F8)
            C50[0:49] = C49
            Cg = np.ascontiguousarray(
                C50.reshape(25, 2, 128, 2048).transpose(0, 2, 1, 3)
                .reshape(25, 128, 4096))
            Cs.append(Cg)
            cm = np.zeros((1, G), np.float16)
            if USE_P2P:
                cm[0, 0:GB] = counts[k * GB:(k + 1) * GB]
            else:
                cm[0, k * GB:(k + 1) * GB] = counts[k * GB:(k + 1) * GB]
            cnts.append(cm)
        return Ts, Cs, cnts

    Ta, Ca, cnta = graph_structs(inputs["ast_edge"], inputs["ast_type"],
                                 inputs["ast_batch"], cfg.NTA2, cfg.NTA)
    Tc, Cc, cntc = graph_structs(inputs["cfg_edge"], inputs["cfg_type"],
                                 inputs["cfg_batch"], cfg.NTC2, cfg.NTC)

    f32 = lambda x: np.ascontiguousarray(np.asarray(x, np.float32))
    f16 = lambda x: np.ascontiguousarray(np.asarray(x, np.float32).astype(np.float16))
    embT_ast = np.zeros((64, cfg.NTA2), np.float32)
    embT_ast[:, 0:cfg.NTA] = f32(inputs["ast_emb"]).T
    embT_cfg = np.zeros((64, cfg.NTC2), np.float32)
    embT_cfg[:, 0:cfg.NTC] = f32(inputs["cfg_emb"]).T
    semT = f32(inputs["struct_sem"]).T.copy()  # [SEM, G]

    ln_g = f32(inputs["ln_g"])
    ln_b = f32(inputs["ln_b"])
    Wc = f32(inputs["Wc"])
    Wcph = (ln_g[:, None] * Wc).astype(np.float16)
    W2blk = np.zeros((128, 128), np.float16)
    W2blk[0:64, 0:64] = f16(inputs["ast_W2"])
    W2blk[64:128, 64:128] = f16(inputs["cfg_W2"])
    bcp = (ln_b @ Wc + f32(inputs["bc"])).reshape(2, 1)
    DDm = np.zeros((128, 64), np.float16)
    DHm = np.zeros((128, 64), np.float16)
    DDm[0:64] = np.eye(64, dtype=np.float16)
    DDm[64:128] = -np.eye(64, dtype=np.float16)
    DHm[64:128] = np.eye(64, dtype=np.float16)

    shared = {
        "embT_ast": embT_ast, "embT_cfg": embT_cfg,
        "astW1": f32(inputs["ast_W1"]), "cfgW1": f32(inputs["cfg_W1"]),
        "astb1h": f16(inputs["ast_b1"]).reshape(1, 64),
        "cfgb1h": f16(inputs["cfg_b1"]).reshape(1, 64),
        "W2blk": W2blk,
        "astb2h": f16(inputs["ast_b2"]).reshape(1, 64),
        "cfgb2h": f16(inputs["cfg_b2"]).reshape(1, 64),
        "Wg1h": f16(inputs["Wg1"]), "bg1c": f32(inputs["bg1"]).reshape(64, 1),
        "Wsemh": f16(inputs["Wsem"]), "bsemc": f32(inputs["bsem"]).reshape(64, 1),
        "Wg2h": f16(inputs["Wg2"]), "bg2c": f32(inputs["bg2"]).reshape(64, 1),
        "Wcph": np.ascontiguousarray(Wcph), "bcp": np.ascontiguousarray(bcp),
        "DD": DDm, "DH": DHm,
    }

    def pack_T(T, nt2):
        # [nt2, NBPT] -> granule layout [13, rows, 1024]
        Tg = np.ascontiguousarray(
            T.reshape(nt2, cfg.TGRAN, 1024).transpose(1, 0, 2))
        return Tg

    in_maps = []
    for k in range(cfg.NCORES):
        m = dict(shared)
        Tg_a = pack_T(Ta[k], cfg.NTA2)
        m["T_a0"] = np.ascontiguousarray(Tg_a[:, 0:128])
        m["T_a1"] = np.ascontiguousarray(Tg_a[:, 128:202])
        m["T_c"] = pack_T(Tc[k], cfg.NTC2)[:, 0:102]
        m["T_c"] = np.ascontiguousarray(m["T_c"])
        m["C_ast"] = Ca[k]
        m["C_cfg"] = Cc[k]
        m["cnt_ast"] = cnta[k]
        m["cnt_cfg"] = cntc[k]
        m["semTh"] = np.ascontiguousarray(
            semT[:, k * GB:(k + 1) * GB].astype(np.float16))
        in_maps.append(m)
    return in_maps


def postprocess(results, cfg: CFG):
    outs = [np.asarray(results[k]["outT"]) for k in range(cfg.NCORES)]
    return np.concatenate(outs, axis=1).T.copy()  # [G, 2]


_CACHED = {}


def kernel(**inputs):
    from concourse.bass_utils import run_bass_kernel_spmd
    cfg = CFG()
    if "nc" not in _CACHED:
        _CACHED["nc"] = build_nc(cfg)
    in_maps = preprocess(inputs, cfg)
    res = run_bass_kernel_spmd(_CACHED["nc"], in_maps,
                               core_ids=list(range(cfg.NCORES)))
    return postprocess(res.results, cfg)
